# revision 16
# baseline (speedup 1.0000x reference)
"""Mixture-of-Experts (B=4, S=2048, D=1024, F=4096, E=8, top-2) on 8 trn2 NeuronCores.

Strategy: expert parallelism, one expert per core.
  - Host: gate (softmax + top-2 + renorm) in float64, dispatch (gather) tokens
    per expert, pad to a common capacity C, pack all device tensors so every
    DMA moves 8-16KB contiguous per SBUF partition (big-packet DMA).
  - Device (SPMD, identical program, per-core data): y^T = W2^T @ gelu(W1^T @ x^T + b1) + b2
    with both weights resident in SBUF as bf16 and tokens streamed in chunks
    of 512. PSUM accumulates over the contraction (D resp. F) in fp32.
    A paced run of warmup matmuls on a memset tile covers the initial DMA
    window so the PE HAM clock-gate is already at 8/8 when real matmuls start.
  - Host: combine with the gate weights (y *= cw) and scatter-add back into
    the [B*S, D] output. Token index sets are unique per expert, so fancy-index
    add per expert is race-free.
"""

import copy
import sys

import numpy as np

for _p in ("/opt/trn_rl_repo", "/opt/pypackages"):
    if _p not in sys.path:
        sys.path.append(_p)

import ml_dtypes

B, S, D = 4, 2048, 1024
F = 4 * D
E = 8
TOP_K = 2
P = 128
CC = 512           # token chunk (free dim of matmuls; PSUM bank = 512 fp32)
KO = D // P        # 8  k-subtiles for the first matmul
FT = F // P        # 32 f-tiles (partition tiles of h)
DT = D // P        # 8  d-tiles (partition tiles of y)
FBLK = 512         # W1 wave width (f-columns per wave)
FB = F // FBLK     # 8 waves
G = 4              # W2 batches
GO = FT // G       # 8 fo-tiles per batch
WARM_N = 68        # PE warmup matmuls (HAM un-throttle) during startup DMA

# test-harness hooks (left off for grading)
TRACE = False
LAST_RESULTS = None

_compiled = {}


def _split_drain_waits(nc, max_waits=1):
    """This walrus build rejects instructions carrying more than one sync
    wait ("Too many sync wait commands"). Keep one wait on the instruction and
    move the excess onto NoOps inserted right before it on the same engine
    (engines are in-order, so blocking semantics are identical). Updates stay
    on the original instruction — moving them to a trailing NoOp could signal
    before the op's writes land."""
    import concourse.mybir as mybir

    m = nc.m
    new_module = copy.replace(m, functions=[])
    for function in m.functions:
        new_function = copy.replace(function, blocks=[])
        new_function.set_allocations_from_list(function.allocations)
        for block in function.blocks:
            out = []
            for inst in block.instructions:
                si = getattr(inst, "sync_info", None)
                on_wait = list(si.on_wait) if si is not None and si.on_wait else []
                if len(on_wait) > max_waits:
                    engine = getattr(inst, "engine", None)
                    extra, keep = on_wait[max_waits:], on_wait[:max_waits]
                    for j, w in enumerate(extra):
                        out.append(
                            mybir.InstNoOp(
                                name=f"{inst.name}-w{j}",
                                engine=engine,
                                sync_info=mybir.SyncInfo(on_wait=[w], on_update=[]),
                                bass_nofuse=True,
                            )
                        )
                    inst.sync_info = mybir.SyncInfo(
                        on_wait=keep,
                        on_update=list(si.on_update) if si.on_update else [],
                    )
                out.append(inst)
            new_function.blocks.append(copy.replace(block, instructions=out))
        new_module.functions.append(new_function)
    nc.m = new_module
    return nc


def _build_nc(C):
    import concourse.bass as bass
    import concourse.mybir as mybir
    from concourse.tile import TileContext

    fp32 = mybir.dt.float32
    bf16 = mybir.dt.bfloat16
    AF = mybir.ActivationFunctionType

    nch = -(-C // CC)
    chunks = [(i * CC, min(CC, C - i * CC)) for i in range(nch)]

    nc = bass.Bass()
    # All DRAM tensors are packed host-side so that one SBUF partition's data
    # is contiguous in DRAM (8-16KB per partition per transfer).
    xp = nc.declare_dram_parameter("xp", [nch, P, KO * CC], bf16, isOutput=False)
    w1p = nc.declare_dram_parameter("w1p", [FB, P, KO * FBLK], bf16, isOutput=False)
    w2p = nc.declare_dram_parameter("w2p", [G, P, GO * D], bf16, isOutput=False)
    b1 = nc.declare_dram_parameter("b1", [P, FT], fp32, isOutput=False)
    b2 = nc.declare_dram_parameter("b2", [P, DT], fp32, isOutput=False)
    yp = nc.declare_dram_parameter("yp", [nch, P, DT * CC], bf16, isOutput=True)

    with TileContext(nc) as tc:
        with (
            tc.tile_pool(name="wpool", bufs=1) as wpool,
            tc.tile_pool(name="xpool", bufs=2) as xpool,
            tc.tile_pool(name="hpool", bufs=1) as hpool,
            tc.tile_pool(name="ypool", bufs=2) as ypool,
            tc.tile_pool(name="hpsum", bufs=4, space="PSUM") as hpsum,
            tc.tile_pool(name="ypsum", bufs=4, space="PSUM") as ypsum,
        ):
            # DMA queue order == program order: chunk-0 activations + the
            # first W1 wave in half-transfers (the blockers for the first
            # real matmul), then the rest of the weights in deadline order.
            # The warm tile is memset on-device so the PE warmup loop can
            # start during the DMA ramp without waiting on any transfer.
            warm_sb = wpool.tile([P, P], bf16, tag="warm")
            nc.vector.memset(warm_sb[:], 0.0)

            # chunk-0 x and W1-wave-0 arrive in ko-pair quarters so the first
            # mm1 chain can start after only ~0.5MB has landed.
            x_sb = [None] * nch
            x_sb[0] = xpool.tile([P, KO * CC], bf16, tag="x", name="x0")
            w1_sb = [None] * FB
            w1_sb[0] = wpool.tile([P, KO * FBLK], bf16, tag="w1w0", name="w1s0")
            for j in range(4):
                nc.sync.dma_start(
                    x_sb[0][:, 2 * j * CC: 2 * (j + 1) * CC],
                    xp[0, :, 2 * j * CC: 2 * (j + 1) * CC])
                nc.sync.dma_start(
                    w1_sb[0][:, 2 * j * FBLK: 2 * (j + 1) * FBLK],
                    w1p[0, :, 2 * j * FBLK: 2 * (j + 1) * FBLK])
                if j == 0:
                    b1_sb = wpool.tile([P, FT], fp32, tag="b1")
                    nc.sync.dma_start(b1_sb[:], b1[:])

            b2_sb = wpool.tile([P, DT], fp32, tag="b2")
            nc.sync.dma_start(b2_sb[:], b2[:])

            for fb in range(1, FB):
                w1_sb[fb] = wpool.tile([P, KO * FBLK], bf16, tag=f"w1w{fb}",
                                       name=f"w1s{fb}")
                nc.sync.dma_start(w1_sb[fb][:], w1p[fb])
            w2_sb = [None] * G
            for g in range(G):
                w2_sb[g] = wpool.tile([P, GO * D], bf16, tag=f"w2g{g}",
                                      name=f"w2s{g}")
                nc.sync.dma_start(w2_sb[g][:], w2p[g])

            # PE warmup: full-size matmuls on the memset tile (no DMA deps)
            # keep the PE busy through the HAM activity window so the real
            # matmuls start at the full 2.4 GHz clock. Count is paced to end
            # roughly when the chunk-0 activations/weights land.
            warm_ps = hpsum.tile([P, CC], fp32, tag="hps")
            for _ in range(WARM_N):
                nc.tensor.matmul(
                    warm_ps[:, 0:64], warm_sb[:, 0:P], warm_sb[:, 0:64],
                    start=True, stop=True,
                )

            for ci, (c0, cn) in enumerate(chunks):
                if ci + 1 < nch:
                    x_sb[ci + 1] = xpool.tile([P, KO * CC], bf16, tag="x",
                                              name=f"x{ci + 1}")
                    nc.sync.dma_start(x_sb[ci + 1][:], xp[ci + 1])

                h_sb = hpool.tile([P, FT * CC], bf16, tag="h")
                for ft in range(FT):
                    fb, fc = divmod(ft * P, FBLK)
                    h_ps = hpsum.tile([P, CC], fp32, tag="hps")
                    for ko in range(KO):
                        nc.tensor.matmul(
                            h_ps[:, :cn],
                            w1_sb[fb][:, ko * FBLK + fc: ko * FBLK + fc + P],
                            x_sb[ci][:, ko * CC: ko * CC + cn],
                            start=(ko == 0),
                            stop=(ko == KO - 1),
                        )
                    # gelu(mm + b1) fused on ScalarE, cast to bf16 on write
                    nc.scalar.activation(
                        h_sb[:, ft * CC: ft * CC + cn], h_ps[:, :cn], AF.Gelu,
                        bias=b1_sb[:, ft:ft + 1],
                    )

                y_sb = ypool.tile([P, DT * CC], bf16, tag="y")
                for dt_ in range(DT):
                    y_ps = ypsum.tile([P, CC], fp32, tag="yps")
                    for fo in range(FT):
                        g, gl = divmod(fo, GO)
                        nc.tensor.matmul(
                            y_ps[:, :cn],
                            w2_sb[g][:, gl * D + dt_ * P: gl * D + dt_ * P + P],
                            h_sb[:, fo * CC: fo * CC + cn],
                            start=(fo == 0),
                            stop=(fo == FT - 1),
                        )
                    nc.vector.tensor_scalar_add(
                        y_sb[:, dt_ * CC: dt_ * CC + cn], y_ps[:, :cn],
                        b2_sb[:, dt_:dt_ + 1],
                    )
                if cn == CC:
                    nc.sync.dma_start(yp[ci], y_sb[:])
                else:
                    for dt_ in range(DT):
                        nc.sync.dma_start(
                            yp[ci, :, dt_ * CC: dt_ * CC + cn],
                            y_sb[:, dt_ * CC: dt_ * CC + cn],
                        )

    return _split_drain_waits(nc)


def _to_bf16(a):
    """Fast float32 -> bfloat16 with round-to-nearest-even via bit ops."""
    a = np.ascontiguousarray(a, dtype=np.float32)
    u = a.view(np.uint32)
    r = ((u + 0x7FFF + ((u >> 16) & 1)) >> 16).astype(np.uint16)
    return r.view(ml_dtypes.bfloat16)


def kernel(hidden_states, Wg, bg, W1, b1, W2, b2):
    from concourse import bass_utils

    hs = np.ascontiguousarray(hidden_states, dtype=np.float32).reshape(B * S, D)

    # ---- Gate on host (float64): softmax over experts, top-2, renormalize
    logits = hs.astype(np.float64) @ np.asarray(Wg, np.float64).T
    logits += np.asarray(bg, np.float64)
    logits -= logits.max(axis=-1, keepdims=True)
    p = np.exp(logits)
    p /= p.sum(axis=-1, keepdims=True)

    i1 = p.argmax(axis=-1)
    rows = np.arange(B * S)
    p1 = p[rows, i1]
    pm = p.copy()
    pm[rows, i1] = -1.0
    i2 = pm.argmax(axis=-1)
    p2 = p[rows, i2]
    denom = p1 + p2
    g1 = (p1 / denom).astype(np.float32)
    g2 = (p2 / denom).astype(np.float32)

    # ---- Dispatch: token ids + combine weights per expert
    ids, cws = [], []
    for e in range(E):
        m1 = np.nonzero(i1 == e)[0]
        m2 = np.nonzero(i2 == e)[0]
        ids.append(np.concatenate([m1, m2]))
        cws.append(np.concatenate([g1[m1], g2[m2]]))
    max_cnt = max(len(x) for x in ids)
    C = max(P, -(-max_cnt // P) * P)
    nch = -(-C // CC)

    if C not in _compiled:
        _compiled[C] = _build_nc(C)
    nc = _compiled[C]

    in_maps = []
    for e in range(E):
        xT = np.zeros((D, nch * CC), dtype=ml_dtypes.bfloat16)
        cnt = len(ids[e])
        xT[:, :cnt] = _to_bf16(hs[ids[e]]).T
        # pack: xp[ch, ki, ko*CC + c'] = xT[ko*P + ki, ch*CC + c']
        xpk = np.ascontiguousarray(
            xT.reshape(KO, P, nch, CC).transpose(2, 1, 0, 3).reshape(nch, P, KO * CC))
        # pack: w1p[fb, ki, ko*FBLK + f'] = W1[ko*P + ki, fb*FBLK + f']
        w1pk = np.ascontiguousarray(
            _to_bf16(W1[e]).reshape(KO, P, FB, FBLK)
            .transpose(2, 1, 0, 3).reshape(FB, P, KO * FBLK))
        # pack: w2p[g, fi, gl*D + d] = W2[(g*GO + gl)*P + fi, d]
        w2pk = np.ascontiguousarray(
            _to_bf16(W2[e]).reshape(G, GO, P, D)
            .transpose(0, 2, 1, 3).reshape(G, P, GO * D))
        in_maps.append({
            "xp": xpk,
            "w1p": w1pk,
            "w2p": w2pk,
            "b1": np.ascontiguousarray(
                np.asarray(b1[e], np.float32).reshape(FT, P).T),
            "b2": np.ascontiguousarray(
                np.asarray(b2[e], np.float32).reshape(DT, P).T),
        })

    kwargs = {}
    if TRACE:
        import os as _os
        kwargs = dict(trace=True, trace_cores=list(range(E)))
        if _os.environ.get("MOE_TRACE_DIR"):
            _os.makedirs(_os.environ["MOE_TRACE_DIR"], exist_ok=True)
            kwargs["tmpdir"] = _os.environ["MOE_TRACE_DIR"]
    res = bass_utils.run_bass_kernel_spmd(nc, in_maps, list(range(E)), **kwargs)
    global LAST_RESULTS
    LAST_RESULTS = res

    out = np.zeros((B * S, D), dtype=np.float32)
    for e in range(E):
        cnt = len(ids[e])
        # unpack: yp[ch, p, dt*CC + c'] = y[dt*P + p, ch*CC + c']
        ypk = np.asarray(res.results[e]["yp"], dtype=np.float32)
        yT = ypk.reshape(nch, P, DT, CC).transpose(2, 1, 0, 3).reshape(D, nch * CC)
        out[ids[e]] += cws[e][:, None] * yT[:, :cnt].T
    return out.reshape(B, S, D)


# revision 17
# speedup vs baseline: 1.2063x; 1.2063x over previous
"""Mixture-of-Experts (B=4, S=2048, D=1024, F=4096, E=8, top-2) on 8 trn2 NeuronCores.

Strategy: F-sharded expert pairs (tensor parallelism inside expert parallelism).
  - Experts are split into a "slot A" set (4 largest token counts) and a
    "slot B" set (4 smallest). Pair p = (A_p, B_p) is served by cores 2p and
    2p+1, each holding HALF of the F dimension of BOTH experts' weights.
    Every core therefore processes C_A + C_B columns at F/2 work per column;
    SPMD capacity padding averages across the pair, so the per-core matmul
    issue time is ~3% lower than one-expert-per-core at max-expert capacity.
  - Host: gate (softmax + top-2 + renorm) in float64, dispatch (gather) tokens
    per expert, pack all device tensors so every DMA moves 8-16KB contiguous
    per SBUF partition (big-packet DMA).
  - Device (SPMD, identical program, per-core data): for each of the two
    expert phases: y_half^T = W2h^T @ gelu(W1h^T @ x^T + b1h) + b2/2 with
    weights resident in SBUF as bf16 and tokens streamed in chunks of 512.
    PSUM accumulates contractions (D resp. F/2) in fp32. A paced run of
    warmup matmuls on a memset tile covers the initial DMA window so the PE
    HAM clock-gate is already at 8/8 when real matmuls start.
  - Host: sum the two partial-y halves per expert, combine with the gate
    weights (y *= cw), scatter-add into the [B*S, D] output. Token index sets
    are unique per expert, so fancy-index add per expert is race-free.
"""

import copy
import sys

import numpy as np

for _p in ("/opt/trn_rl_repo", "/opt/pypackages"):
    if _p not in sys.path:
        sys.path.append(_p)

import ml_dtypes

B, S, D = 4, 2048, 1024
F = 4 * D
E = 8
TOP_K = 2
P = 128
CC = 512           # token chunk (free dim of matmuls; PSUM bank = 512 fp32)
KO = D // P        # 8  k-subtiles for the first matmul
FH = F // 2        # 2048 f-columns handled per core per expert
FT = FH // P       # 16 f-tiles (partition tiles of h) per phase
DT = D // P        # 8  d-tiles (partition tiles of y)
FBLK = 512         # W1 wave width (f-columns per wave)
FB = FH // FBLK    # 4 waves per phase (8 total)
GO = 8             # fo-tiles per W2 batch (2 batches per phase, 4 total)
WARM_N = 68        # PE warmup matmuls (HAM un-throttle) during startup DMA

# test-harness hooks (left off for grading)
TRACE = False
LAST_RESULTS = None

_compiled = {}


def _split_drain_waits(nc, max_waits=1):
    """This walrus build rejects instructions carrying more than one sync
    wait ("Too many sync wait commands"). Keep one wait on the instruction and
    move the excess onto NoOps inserted right before it on the same engine
    (engines are in-order, so blocking semantics are identical). Updates stay
    on the original instruction — moving them to a trailing NoOp could signal
    before the op's writes land."""
    import concourse.mybir as mybir

    m = nc.m
    new_module = copy.replace(m, functions=[])
    for function in m.functions:
        new_function = copy.replace(function, blocks=[])
        new_function.set_allocations_from_list(function.allocations)
        for block in function.blocks:
            out = []
            for inst in block.instructions:
                si = getattr(inst, "sync_info", None)
                on_wait = list(si.on_wait) if si is not None and si.on_wait else []
                if len(on_wait) > max_waits:
                    engine = getattr(inst, "engine", None)
                    extra, keep = on_wait[max_waits:], on_wait[:max_waits]
                    for j, w in enumerate(extra):
                        out.append(
                            mybir.InstNoOp(
                                name=f"{inst.name}-w{j}",
                                engine=engine,
                                sync_info=mybir.SyncInfo(on_wait=[w], on_update=[]),
                                bass_nofuse=True,
                            )
                        )
                    inst.sync_info = mybir.SyncInfo(
                        on_wait=keep,
                        on_update=list(si.on_update) if si.on_update else [],
                    )
                out.append(inst)
            new_function.blocks.append(copy.replace(block, instructions=out))
        new_module.functions.append(new_function)
    nc.m = new_module
    return nc


def _chunk_list(CA, CB):
    """Global chunk descriptors: (phase, c0_in_phase, cn)."""
    chunks = []
    for ph, C in ((0, CA), (1, CB)):
        c0 = 0
        while c0 < C:
            chunks.append((ph, c0, min(CC, C - c0)))
            c0 += CC
    return chunks


def _build_nc(CA, CB):
    import concourse.bass as bass
    import concourse.mybir as mybir
    from concourse.tile import TileContext

    fp32 = mybir.dt.float32
    bf16 = mybir.dt.bfloat16
    AF = mybir.ActivationFunctionType

    chunks = _chunk_list(CA, CB)
    nch = len(chunks)

    nc = bass.Bass()
    # All DRAM tensors are packed host-side so that one SBUF partition's data
    # is contiguous in DRAM (8-16KB per partition per transfer).
    # Waves 0-3 / W2 batches 0-1 / b-cols 0-15 belong to expert phase A;
    # waves 4-7 / batches 2-3 / b-cols 16-31 to phase B.
    xp = nc.declare_dram_parameter("xp", [nch, P, KO * CC], bf16, isOutput=False)
    w1p = nc.declare_dram_parameter("w1p", [2 * FB, P, KO * FBLK], bf16,
                                    isOutput=False)
    w2p = nc.declare_dram_parameter("w2p", [4, P, GO * D], bf16, isOutput=False)
    b1 = nc.declare_dram_parameter("b1", [P, 2 * FT], fp32, isOutput=False)
    b2 = nc.declare_dram_parameter("b2", [P, 2 * DT], fp32, isOutput=False)
    yp = nc.declare_dram_parameter("yp", [nch, P, DT * CC], bf16, isOutput=True)

    with TileContext(nc) as tc:
        with (
            tc.tile_pool(name="wpool", bufs=1) as wpool,
            tc.tile_pool(name="xpool", bufs=2) as xpool,
            tc.tile_pool(name="hpool", bufs=1) as hpool,
            tc.tile_pool(name="ypool", bufs=2) as ypool,
            tc.tile_pool(name="hpsum", bufs=4, space="PSUM") as hpsum,
            tc.tile_pool(name="ypsum", bufs=4, space="PSUM") as ypsum,
        ):
            # The warm tile is memset on-device so the PE warmup loop can
            # start during the DMA ramp without waiting on any transfer.
            warm_sb = wpool.tile([P, P], bf16, tag="warm")
            nc.vector.memset(warm_sb[:], 0.0)

            # DMA ring order == program order: chunk-0 x and W1-wave-0 arrive
            # in ko-pair quarters so the first mm1 chain can start after only
            # ~0.5MB has landed; then the rest in deadline order.
            x_sb = [None] * nch
            x_sb[0] = xpool.tile([P, KO * CC], bf16, tag="x", name="x0")
            w1_sb = [None] * (2 * FB)
            w1_sb[0] = wpool.tile([P, KO * FBLK], bf16, tag="w1w0", name="w1s0")
            for j in range(4):
                nc.sync.dma_start(
                    x_sb[0][:, 2 * j * CC: 2 * (j + 1) * CC],
                    xp[0, :, 2 * j * CC: 2 * (j + 1) * CC])
                nc.sync.dma_start(
                    w1_sb[0][:, 2 * j * FBLK: 2 * (j + 1) * FBLK],
                    w1p[0, :, 2 * j * FBLK: 2 * (j + 1) * FBLK])
                if j == 0:
                    b1_sb = wpool.tile([P, 2 * FT], fp32, tag="b1")
                    nc.sync.dma_start(b1_sb[:], b1[:])

            b2_sb = wpool.tile([P, 2 * DT], fp32, tag="b2")
            nc.sync.dma_start(b2_sb[:], b2[:])

            for fb in range(1, 2 * FB):
                w1_sb[fb] = wpool.tile([P, KO * FBLK], bf16, tag=f"w1w{fb}",
                                       name=f"w1s{fb}")
                nc.sync.dma_start(w1_sb[fb][:], w1p[fb])
            w2_sb = [None] * 4
            for g in range(4):
                w2_sb[g] = wpool.tile([P, GO * D], bf16, tag=f"w2g{g}",
                                      name=f"w2s{g}")
                nc.sync.dma_start(w2_sb[g][:], w2p[g])

            # PE warmup: matmuls on the memset tile (no DMA deps) keep the PE
            # busy through the HAM activity window so the real matmuls start
            # at the full 2.4 GHz clock. Count is paced to end roughly when
            # the chunk-0 activations/weights land.
            warm_ps = hpsum.tile([P, CC], fp32, tag="hps")
            for _ in range(WARM_N):
                nc.tensor.matmul(
                    warm_ps[:, 0:64], warm_sb[:, 0:P], warm_sb[:, 0:64],
                    start=True, stop=True,
                )

            for ci, (ph, c0, cn) in enumerate(chunks):
                if ci + 1 < nch:
                    x_sb[ci + 1] = xpool.tile([P, KO * CC], bf16, tag="x",
                                              name=f"x{ci + 1}")
                    nc.sync.dma_start(x_sb[ci + 1][:], xp[ci + 1])

                h_sb = hpool.tile([P, FT * CC], bf16, tag="h")
                for ft in range(FT):
                    fb, fc = divmod(ft * P, FBLK)
                    fb += ph * FB
                    h_ps = hpsum.tile([P, CC], fp32, tag="hps")
                    for ko in range(KO):
                        nc.tensor.matmul(
                            h_ps[:, :cn],
                            w1_sb[fb][:, ko * FBLK + fc: ko * FBLK + fc + P],
                            x_sb[ci][:, ko * CC: ko * CC + cn],
                            start=(ko == 0),
                            stop=(ko == KO - 1),
                        )
                    # gelu(mm + b1) fused on ScalarE, cast to bf16 on write
                    nc.scalar.activation(
                        h_sb[:, ft * CC: ft * CC + cn], h_ps[:, :cn], AF.Gelu,
                        bias=b1_sb[:, ph * FT + ft: ph * FT + ft + 1],
                    )

                y_sb = ypool.tile([P, DT * CC], bf16, tag="y")
                for dt_ in range(DT):
                    y_ps = ypsum.tile([P, CC], fp32, tag="yps")
                    for fo in range(FT):
                        g = 2 * ph + fo // GO
                        gl = fo % GO
                        nc.tensor.matmul(
                            y_ps[:, :cn],
                            w2_sb[g][:, gl * D + dt_ * P: gl * D + dt_ * P + P],
                            h_sb[:, fo * CC: fo * CC + cn],
                            start=(fo == 0),
                            stop=(fo == FT - 1),
                        )
                    nc.vector.tensor_scalar_add(
                        y_sb[:, dt_ * CC: dt_ * CC + cn], y_ps[:, :cn],
                        b2_sb[:, ph * DT + dt_: ph * DT + dt_ + 1],
                    )
                if cn == CC:
                    nc.sync.dma_start(yp[ci], y_sb[:])
                else:
                    for dt_ in range(DT):
                        nc.sync.dma_start(
                            yp[ci, :, dt_ * CC: dt_ * CC + cn],
                            y_sb[:, dt_ * CC: dt_ * CC + cn],
                        )

    return _split_drain_waits(nc)


def _to_bf16(a):
    """Fast float32 -> bfloat16 with round-to-nearest-even via bit ops."""
    a = np.ascontiguousarray(a, dtype=np.float32)
    u = a.view(np.uint32)
    r = ((u + 0x7FFF + ((u >> 16) & 1)) >> 16).astype(np.uint16)
    return r.view(ml_dtypes.bfloat16)


def _pack_x(hs_rows, C, nchp):
    """[cnt, D] tokens -> [nchp, P, KO*CC] zero-padded chunk tiles."""
    xT = np.zeros((D, nchp * CC), dtype=ml_dtypes.bfloat16)
    xT[:, :hs_rows.shape[0]] = _to_bf16(hs_rows).T
    return xT.reshape(KO, P, nchp, CC).transpose(2, 1, 0, 3).reshape(
        nchp, P, KO * CC)


def _pack_w1_half(W1e, h):
    """W1[e] [D, F] F-half h -> [FB, P, KO*FBLK] wave tiles."""
    half = np.ascontiguousarray(W1e[:, h * FH:(h + 1) * FH])
    return np.ascontiguousarray(
        _to_bf16(half).reshape(KO, P, FB, FBLK)
        .transpose(2, 1, 0, 3).reshape(FB, P, KO * FBLK))


def _pack_w2_half(W2e, h):
    """W2[e] [F, D] row-half h -> [2, P, GO*D] batch tiles."""
    half = np.ascontiguousarray(W2e[h * FH:(h + 1) * FH, :])
    return np.ascontiguousarray(
        _to_bf16(half).reshape(2, GO, P, D)
        .transpose(0, 2, 1, 3).reshape(2, P, GO * D))


def kernel(hidden_states, Wg, bg, W1, b1, W2, b2):
    from concourse import bass_utils

    hs = np.ascontiguousarray(hidden_states, dtype=np.float32).reshape(B * S, D)

    # ---- Gate on host (float64): softmax over experts, top-2, renormalize
    logits = hs.astype(np.float64) @ np.asarray(Wg, np.float64).T
    logits += np.asarray(bg, np.float64)
    logits -= logits.max(axis=-1, keepdims=True)
    p = np.exp(logits)
    p /= p.sum(axis=-1, keepdims=True)

    i1 = p.argmax(axis=-1)
    rows = np.arange(B * S)
    p1 = p[rows, i1]
    pm = p.copy()
    pm[rows, i1] = -1.0
    i2 = pm.argmax(axis=-1)
    p2 = p[rows, i2]
    denom = p1 + p2
    g1 = (p1 / denom).astype(np.float32)
    g2 = (p2 / denom).astype(np.float32)

    # ---- Dispatch: token ids + combine weights per expert
    ids, cws = [], []
    for e in range(E):
        m1 = np.nonzero(i1 == e)[0]
        m2 = np.nonzero(i2 == e)[0]
        ids.append(np.concatenate([m1, m2]))
        cws.append(np.concatenate([g1[m1], g2[m2]]))

    # Slot A = the 4 busiest experts, slot B = the 4 least busy; pair A_p
    # with B_p. Phase capacities are the max count within each slot.
    order = sorted(range(E), key=lambda e: -len(ids[e]))
    slotA, slotB = order[:4], order[4:]
    CA = max(P, -(-max(len(ids[e]) for e in slotA) // P) * P)
    CB = max(P, -(-max(len(ids[e]) for e in slotB) // P) * P)
    nchA, nchB = -(-CA // CC), -(-CB // CC)
    nch = nchA + nchB

    if (CA, CB) not in _compiled:
        _compiled[(CA, CB)] = _build_nc(CA, CB)
    nc = _compiled[(CA, CB)]

    in_maps = []
    for pr in range(4):
        ea, eb = slotA[pr], slotB[pr]
        xpk = np.concatenate(
            [_pack_x(hs[ids[ea]], CA, nchA), _pack_x(hs[ids[eb]], CB, nchB)])
        for h in range(2):
            w1pk = np.concatenate(
                [_pack_w1_half(np.asarray(W1[ea]), h),
                 _pack_w1_half(np.asarray(W1[eb]), h)])
            w2pk = np.concatenate(
                [_pack_w2_half(np.asarray(W2[ea]), h),
                 _pack_w2_half(np.asarray(W2[eb]), h)])
            b1pk = np.concatenate(
                [np.asarray(b1[ea], np.float32)[h * FH:(h + 1) * FH]
                 .reshape(FT, P).T,
                 np.asarray(b1[eb], np.float32)[h * FH:(h + 1) * FH]
                 .reshape(FT, P).T], axis=1)
            b2pk = np.concatenate(
                [np.asarray(b2[ea], np.float32).reshape(DT, P).T * 0.5,
                 np.asarray(b2[eb], np.float32).reshape(DT, P).T * 0.5], axis=1)
            in_maps.append({
                "xp": xpk,
                "w1p": w1pk,
                "w2p": w2pk,
                "b1": np.ascontiguousarray(b1pk),
                "b2": np.ascontiguousarray(b2pk),
            })

    kwargs = {}
    if TRACE:
        import os as _os
        kwargs = dict(trace=True, trace_cores=list(range(E)))
        if _os.environ.get("MOE_TRACE_DIR"):
            _os.makedirs(_os.environ["MOE_TRACE_DIR"], exist_ok=True)
            kwargs["tmpdir"] = _os.environ["MOE_TRACE_DIR"]
    res = bass_utils.run_bass_kernel_spmd(nc, in_maps, list(range(E)), **kwargs)
    global LAST_RESULTS
    LAST_RESULTS = res

    def unpack(ypk, lo, nchp):
        # yp[ch, p, dt*CC + c'] = y[dt*P + p, ch*CC + c']
        blk = ypk[lo:lo + nchp]
        return blk.reshape(nchp, P, DT, CC).transpose(2, 1, 0, 3).reshape(
            D, nchp * CC)

    out = np.zeros((B * S, D), dtype=np.float32)
    for pr in range(4):
        y0 = np.asarray(res.results[2 * pr]["yp"], dtype=np.float32)
        y1 = np.asarray(res.results[2 * pr + 1]["yp"], dtype=np.float32)
        for (e, lo, nchp) in ((slotA[pr], 0, nchA), (slotB[pr], nchA, nchB)):
            cnt = len(ids[e])
            yT = unpack(y0, lo, nchp) + unpack(y1, lo, nchp)
            out[ids[e]] += cws[e][:, None] * yT[:, :cnt].T
    return out.reshape(B, S, D)


# revision 19
# speedup vs baseline: 1.2245x; 1.0150x over previous
"""Mixture-of-Experts (B=4, S=2048, D=1024, F=4096, E=8, top-2) on 8 trn2 NeuronCores.

Strategy: F-sharded expert pairs (tensor parallelism inside expert parallelism).
  - Experts are split into a "slot A" set (4 largest token counts) and a
    "slot B" set (4 smallest). Pair p = (A_p, B_p) is served by cores 2p and
    2p+1, each holding HALF of the F dimension of BOTH experts' weights.
    Every core therefore processes C_A + C_B columns at F/2 work per column;
    SPMD capacity padding averages across the pair, so the per-core matmul
    issue time is ~3% lower than one-expert-per-core at max-expert capacity.
  - Host: gate (softmax + top-2 + renorm) in float64, dispatch (gather) tokens
    per expert, pack all device tensors so every DMA moves 8-16KB contiguous
    per SBUF partition (big-packet DMA).
  - Device (SPMD, identical program, per-core data): for each of the two
    expert phases: y_half^T = W2h^T @ gelu(W1h^T @ x^T + b1h) + b2/2 with
    weights resident in SBUF as bf16 and tokens streamed in chunks of 512.
    PSUM accumulates contractions (D resp. F/2) in fp32. A paced run of
    warmup matmuls on a memset tile covers the initial DMA window so the PE
    HAM clock-gate is already at 8/8 when real matmuls start.
  - Host: sum the two partial-y halves per expert, combine with the gate
    weights (y *= cw), scatter-add into the [B*S, D] output. Token index sets
    are unique per expert, so fancy-index add per expert is race-free.
"""

import copy
import sys

import numpy as np

for _p in ("/opt/trn_rl_repo", "/opt/pypackages"):
    if _p not in sys.path:
        sys.path.append(_p)

import ml_dtypes

B, S, D = 4, 2048, 1024
F = 4 * D
E = 8
TOP_K = 2
P = 128
CC = 512           # token chunk (free dim of matmuls; PSUM bank = 512 fp32)
KO = D // P        # 8  k-subtiles for the first matmul
FH = F // 2        # 2048 f-columns handled per core per expert
FT = FH // P       # 16 f-tiles (partition tiles of h) per phase
DT = D // P        # 8  d-tiles (partition tiles of y)
FBLK = 512         # W1 wave width (f-columns per wave)
FB = FH // FBLK    # 4 waves per phase (8 total)
GO = 8             # fo-tiles per W2 batch (2 batches per phase, 4 total)
WARM_N = 68        # PE warmup matmuls (HAM un-throttle) during startup DMA

# test-harness hooks (left off for grading)
TRACE = False
LAST_RESULTS = None

_compiled = {}


def _split_drain_waits(nc, max_waits=1):
    """This walrus build rejects instructions carrying more than one sync
    wait ("Too many sync wait commands"). Keep one wait on the instruction and
    move the excess onto NoOps inserted right before it on the same engine
    (engines are in-order, so blocking semantics are identical). Updates stay
    on the original instruction — moving them to a trailing NoOp could signal
    before the op's writes land."""
    import concourse.mybir as mybir

    m = nc.m
    new_module = copy.replace(m, functions=[])
    for function in m.functions:
        new_function = copy.replace(function, blocks=[])
        new_function.set_allocations_from_list(function.allocations)
        for block in function.blocks:
            out = []
            for inst in block.instructions:
                si = getattr(inst, "sync_info", None)
                on_wait = list(si.on_wait) if si is not None and si.on_wait else []
                if len(on_wait) > max_waits:
                    engine = getattr(inst, "engine", None)
                    extra, keep = on_wait[max_waits:], on_wait[:max_waits]
                    for j, w in enumerate(extra):
                        out.append(
                            mybir.InstNoOp(
                                name=f"{inst.name}-w{j}",
                                engine=engine,
                                sync_info=mybir.SyncInfo(on_wait=[w], on_update=[]),
                                bass_nofuse=True,
                            )
                        )
                    inst.sync_info = mybir.SyncInfo(
                        on_wait=keep,
                        on_update=list(si.on_update) if si.on_update else [],
                    )
                out.append(inst)
            new_function.blocks.append(copy.replace(block, instructions=out))
        new_module.functions.append(new_function)
    nc.m = new_module
    return nc


def _chunk_list(CA, CB):
    """Global chunk descriptors: (phase, c0_in_phase, cn)."""
    chunks = []
    for ph, C in ((0, CA), (1, CB)):
        c0 = 0
        while c0 < C:
            chunks.append((ph, c0, min(CC, C - c0)))
            c0 += CC
    return chunks


def _build_nc(CA, CB):
    import concourse.bass as bass
    import concourse.mybir as mybir
    from concourse.tile import TileContext

    fp32 = mybir.dt.float32
    bf16 = mybir.dt.bfloat16
    AF = mybir.ActivationFunctionType

    chunks = _chunk_list(CA, CB)
    nch = len(chunks)

    nc = bass.Bass()
    # All DRAM tensors are packed host-side so that one SBUF partition's data
    # is contiguous in DRAM (8-16KB per partition per transfer).
    # Waves 0-3 / W2 batches 0-1 / b-cols 0-15 belong to expert phase A;
    # waves 4-7 / batches 2-3 / b-cols 16-31 to phase B.
    xp = nc.declare_dram_parameter("xp", [nch, P, KO * CC], bf16, isOutput=False)
    w1p = nc.declare_dram_parameter("w1p", [2 * FB, P, KO * FBLK], bf16,
                                    isOutput=False)
    w2p = nc.declare_dram_parameter("w2p", [4, P, GO * D], bf16, isOutput=False)
    b1 = nc.declare_dram_parameter("b1", [P, 2 * FT], fp32, isOutput=False)
    b2 = nc.declare_dram_parameter("b2", [P, 2 * DT], fp32, isOutput=False)
    yp = nc.declare_dram_parameter("yp", [nch, P, DT * CC], bf16, isOutput=True)

    with TileContext(nc) as tc:
        with (
            tc.tile_pool(name="wpool", bufs=1) as wpool,
            tc.tile_pool(name="xpool", bufs=2) as xpool,
            tc.tile_pool(name="hpool", bufs=1) as hpool,
            tc.tile_pool(name="ypool", bufs=2) as ypool,
            tc.tile_pool(name="hpsum", bufs=4, space="PSUM") as hpsum,
            tc.tile_pool(name="ypsum", bufs=4, space="PSUM") as ypsum,
        ):
            # The warm tile is memset on-device so the PE warmup loop can
            # start during the DMA ramp without waiting on any transfer.
            warm_sb = wpool.tile([P, P], bf16, tag="warm")
            nc.vector.memset(warm_sb[:], 0.0)

            # DMA ring order == program order: chunk-0 x and W1-wave-0 arrive
            # in ko-pair quarters so the first mm1 chain can start after only
            # ~0.5MB has landed; then the rest in deadline order.
            x_sb = [None] * nch
            x_sb[0] = xpool.tile([P, KO * CC], bf16, tag="x", name="x0")
            w1_sb = [None] * (2 * FB)
            w1_sb[0] = wpool.tile([P, KO * FBLK], bf16, tag="w1w0", name="w1s0")
            for j in range(4):
                nc.sync.dma_start(
                    x_sb[0][:, 2 * j * CC: 2 * (j + 1) * CC],
                    xp[0, :, 2 * j * CC: 2 * (j + 1) * CC])
                nc.sync.dma_start(
                    w1_sb[0][:, 2 * j * FBLK: 2 * (j + 1) * FBLK],
                    w1p[0, :, 2 * j * FBLK: 2 * (j + 1) * FBLK])
                if j == 0:
                    b1_sb = wpool.tile([P, 2 * FT], fp32, tag="b1")
                    nc.sync.dma_start(b1_sb[:], b1[:])

            b2_sb = wpool.tile([P, 2 * DT], fp32, tag="b2")
            nc.sync.dma_start(b2_sb[:], b2[:])

            # Deadline order: phase-A W1 waves, phase-A W2 batches, then all
            # of phase B (needed only ~235us in).
            w2_sb = [None] * 4
            for fb in list(range(1, FB)) + [None] + list(range(FB, 2 * FB)):
                if fb is None:
                    for g in (0, 1):
                        w2_sb[g] = wpool.tile([P, GO * D], bf16, tag=f"w2g{g}",
                                              name=f"w2s{g}")
                        nc.sync.dma_start(w2_sb[g][:], w2p[g])
                    continue
                w1_sb[fb] = wpool.tile([P, KO * FBLK], bf16, tag=f"w1w{fb}",
                                       name=f"w1s{fb}")
                nc.sync.dma_start(w1_sb[fb][:], w1p[fb])
            for g in (2, 3):
                w2_sb[g] = wpool.tile([P, GO * D], bf16, tag=f"w2g{g}",
                                      name=f"w2s{g}")
                nc.sync.dma_start(w2_sb[g][:], w2p[g])

            # PE warmup: matmuls on the memset tile (no DMA deps) keep the PE
            # busy through the HAM activity window so the real matmuls start
            # at the full 2.4 GHz clock. Count is paced to end roughly when
            # the chunk-0 activations/weights land.
            warm_ps = hpsum.tile([P, CC], fp32, tag="hps")
            for _ in range(WARM_N):
                nc.tensor.matmul(
                    warm_ps[:, 0:64], warm_sb[:, 0:P], warm_sb[:, 0:64],
                    start=True, stop=True,
                )

            for ci, (ph, c0, cn) in enumerate(chunks):
                if ci + 1 < nch:
                    x_sb[ci + 1] = xpool.tile([P, KO * CC], bf16, tag="x",
                                              name=f"x{ci + 1}")
                    nc.sync.dma_start(x_sb[ci + 1][:], xp[ci + 1])

                h_sb = hpool.tile([P, FT * CC], bf16, tag="h")
                for ft in range(FT):
                    fb, fc = divmod(ft * P, FBLK)
                    fb += ph * FB
                    h_ps = hpsum.tile([P, CC], fp32, tag="hps")
                    for ko in range(KO):
                        nc.tensor.matmul(
                            h_ps[:, :cn],
                            w1_sb[fb][:, ko * FBLK + fc: ko * FBLK + fc + P],
                            x_sb[ci][:, ko * CC: ko * CC + cn],
                            start=(ko == 0),
                            stop=(ko == KO - 1),
                        )
                    # gelu(mm + b1) fused on ScalarE, cast to bf16 on write
                    nc.scalar.activation(
                        h_sb[:, ft * CC: ft * CC + cn], h_ps[:, :cn], AF.Gelu,
                        bias=b1_sb[:, ph * FT + ft: ph * FT + ft + 1],
                    )

                y_sb = ypool.tile([P, DT * CC], bf16, tag="y")
                for dt_ in range(DT):
                    y_ps = ypsum.tile([P, CC], fp32, tag="yps")
                    for fo in range(FT):
                        g = 2 * ph + fo // GO
                        gl = fo % GO
                        nc.tensor.matmul(
                            y_ps[:, :cn],
                            w2_sb[g][:, gl * D + dt_ * P: gl * D + dt_ * P + P],
                            h_sb[:, fo * CC: fo * CC + cn],
                            start=(fo == 0),
                            stop=(fo == FT - 1),
                        )
                    nc.vector.tensor_scalar_add(
                        y_sb[:, dt_ * CC: dt_ * CC + cn], y_ps[:, :cn],
                        b2_sb[:, ph * DT + dt_: ph * DT + dt_ + 1],
                    )
                if cn == CC and ci + 1 < nch:
                    nc.sync.dma_start(yp[ci], y_sb[:])
                else:
                    # Partial chunks and the final chunk drain per d-tile so
                    # the output DMA starts as soon as each DVE add lands.
                    for dt_ in range(DT):
                        nc.sync.dma_start(
                            yp[ci, :, dt_ * CC: dt_ * CC + cn],
                            y_sb[:, dt_ * CC: dt_ * CC + cn],
                        )

    return _split_drain_waits(nc)


def _to_bf16(a):
    """Fast float32 -> bfloat16 with round-to-nearest-even via bit ops."""
    a = np.ascontiguousarray(a, dtype=np.float32)
    u = a.view(np.uint32)
    r = ((u + 0x7FFF + ((u >> 16) & 1)) >> 16).astype(np.uint16)
    return r.view(ml_dtypes.bfloat16)


def _pack_x(hs_rows, C, nchp):
    """[cnt, D] tokens -> [nchp, P, KO*CC] zero-padded chunk tiles."""
    xT = np.zeros((D, nchp * CC), dtype=ml_dtypes.bfloat16)
    xT[:, :hs_rows.shape[0]] = _to_bf16(hs_rows).T
    return xT.reshape(KO, P, nchp, CC).transpose(2, 1, 0, 3).reshape(
        nchp, P, KO * CC)


def _pack_w1_half(W1e, h):
    """W1[e] [D, F] F-half h -> [FB, P, KO*FBLK] wave tiles."""
    half = np.ascontiguousarray(W1e[:, h * FH:(h + 1) * FH])
    return np.ascontiguousarray(
        _to_bf16(half).reshape(KO, P, FB, FBLK)
        .transpose(2, 1, 0, 3).reshape(FB, P, KO * FBLK))


def _pack_w2_half(W2e, h):
    """W2[e] [F, D] row-half h -> [2, P, GO*D] batch tiles."""
    half = np.ascontiguousarray(W2e[h * FH:(h + 1) * FH, :])
    return np.ascontiguousarray(
        _to_bf16(half).reshape(2, GO, P, D)
        .transpose(0, 2, 1, 3).reshape(2, P, GO * D))


def kernel(hidden_states, Wg, bg, W1, b1, W2, b2):
    from concourse import bass_utils

    hs = np.ascontiguousarray(hidden_states, dtype=np.float32).reshape(B * S, D)

    # ---- Gate on host (float64): softmax over experts, top-2, renormalize
    logits = hs.astype(np.float64) @ np.asarray(Wg, np.float64).T
    logits += np.asarray(bg, np.float64)
    logits -= logits.max(axis=-1, keepdims=True)
    p = np.exp(logits)
    p /= p.sum(axis=-1, keepdims=True)

    i1 = p.argmax(axis=-1)
    rows = np.arange(B * S)
    p1 = p[rows, i1]
    pm = p.copy()
    pm[rows, i1] = -1.0
    i2 = pm.argmax(axis=-1)
    p2 = p[rows, i2]
    denom = p1 + p2
    g1 = (p1 / denom).astype(np.float32)
    g2 = (p2 / denom).astype(np.float32)

    # ---- Dispatch: token ids + combine weights per expert
    ids, cws = [], []
    for e in range(E):
        m1 = np.nonzero(i1 == e)[0]
        m2 = np.nonzero(i2 == e)[0]
        ids.append(np.concatenate([m1, m2]))
        cws.append(np.concatenate([g1[m1], g2[m2]]))

    # Slot A = the 4 busiest experts, slot B = the 4 least busy; pair A_p
    # with B_p. Phase capacities are the max count within each slot.
    order = sorted(range(E), key=lambda e: -len(ids[e]))
    slotA, slotB = order[:4], order[4:]
    CA = max(P, -(-max(len(ids[e]) for e in slotA) // P) * P)
    CB = max(P, -(-max(len(ids[e]) for e in slotB) // P) * P)
    nchA, nchB = -(-CA // CC), -(-CB // CC)
    nch = nchA + nchB

    if (CA, CB) not in _compiled:
        _compiled[(CA, CB)] = _build_nc(CA, CB)
    nc = _compiled[(CA, CB)]

    in_maps = []
    for pr in range(4):
        ea, eb = slotA[pr], slotB[pr]
        xpk = np.concatenate(
            [_pack_x(hs[ids[ea]], CA, nchA), _pack_x(hs[ids[eb]], CB, nchB)])
        for h in range(2):
            w1pk = np.concatenate(
                [_pack_w1_half(np.asarray(W1[ea]), h),
                 _pack_w1_half(np.asarray(W1[eb]), h)])
            w2pk = np.concatenate(
                [_pack_w2_half(np.asarray(W2[ea]), h),
                 _pack_w2_half(np.asarray(W2[eb]), h)])
            b1pk = np.concatenate(
                [np.asarray(b1[ea], np.float32)[h * FH:(h + 1) * FH]
                 .reshape(FT, P).T,
                 np.asarray(b1[eb], np.float32)[h * FH:(h + 1) * FH]
                 .reshape(FT, P).T], axis=1)
            b2pk = np.concatenate(
                [np.asarray(b2[ea], np.float32).reshape(DT, P).T * 0.5,
                 np.asarray(b2[eb], np.float32).reshape(DT, P).T * 0.5], axis=1)
            in_maps.append({
                "xp": xpk,
                "w1p": w1pk,
                "w2p": w2pk,
                "b1": np.ascontiguousarray(b1pk),
                "b2": np.ascontiguousarray(b2pk),
            })

    kwargs = {}
    if TRACE:
        import os as _os
        kwargs = dict(trace=True, trace_cores=list(range(E)))
        if _os.environ.get("MOE_TRACE_DIR"):
            _os.makedirs(_os.environ["MOE_TRACE_DIR"], exist_ok=True)
            kwargs["tmpdir"] = _os.environ["MOE_TRACE_DIR"]
    res = bass_utils.run_bass_kernel_spmd(nc, in_maps, list(range(E)), **kwargs)
    global LAST_RESULTS
    LAST_RESULTS = res

    def unpack(ypk, lo, nchp):
        # yp[ch, p, dt*CC + c'] = y[dt*P + p, ch*CC + c']
        blk = ypk[lo:lo + nchp]
        return blk.reshape(nchp, P, DT, CC).transpose(2, 1, 0, 3).reshape(
            D, nchp * CC)

    out = np.zeros((B * S, D), dtype=np.float32)
    for pr in range(4):
        y0 = np.asarray(res.results[2 * pr]["yp"], dtype=np.float32)
        y1 = np.asarray(res.results[2 * pr + 1]["yp"], dtype=np.float32)
        for (e, lo, nchp) in ((slotA[pr], 0, nchA), (slotB[pr], nchA, nchB)):
            cnt = len(ids[e])
            yT = unpack(y0, lo, nchp) + unpack(y1, lo, nchp)
            out[ids[e]] += cws[e][:, None] * yT[:, :cnt].T
    return out.reshape(B, S, D)


# revision 21
# speedup vs baseline: 1.2290x; 1.0037x over previous
"""Mixture-of-Experts (B=4, S=2048, D=1024, F=4096, E=8, top-2) on 8 trn2 NeuronCores.

Strategy: F-sharded expert pairs (tensor parallelism inside expert parallelism).
  - Experts are split into a "slot A" set (4 largest token counts) and a
    "slot B" set (4 smallest). Pair p = (A_p, B_p) is served by cores 2p and
    2p+1, each holding HALF of the F dimension of BOTH experts' weights.
    Every core therefore processes C_A + C_B columns at F/2 work per column;
    SPMD capacity padding averages across the pair, so the per-core matmul
    issue time is ~3% lower than one-expert-per-core at max-expert capacity.
  - Host: gate (softmax + top-2 + renorm) in float64, dispatch (gather) tokens
    per expert, pack all device tensors so every DMA moves 8-16KB contiguous
    per SBUF partition (big-packet DMA).
  - Device (SPMD, identical program, per-core data): for each of the two
    expert phases: y_half^T = W2h^T @ gelu(W1h^T @ x^T + b1h) + b2/2 with
    weights resident in SBUF as bf16 and tokens streamed in chunks of 512.
    PSUM accumulates contractions (D resp. F/2) in fp32. A paced run of
    warmup matmuls on a memset tile covers the initial DMA window so the PE
    HAM clock-gate is already at 8/8 when real matmuls start.
  - Host: sum the two partial-y halves per expert, combine with the gate
    weights (y *= cw), scatter-add into the [B*S, D] output. Token index sets
    are unique per expert, so fancy-index add per expert is race-free.
"""

import copy
import sys

import numpy as np

for _p in ("/opt/trn_rl_repo", "/opt/pypackages"):
    if _p not in sys.path:
        sys.path.append(_p)

import ml_dtypes

B, S, D = 4, 2048, 1024
F = 4 * D
E = 8
TOP_K = 2
P = 128
CC = 512           # token chunk (free dim of matmuls; PSUM bank = 512 fp32)
KO = D // P        # 8  k-subtiles for the first matmul
FH = F // 2        # 2048 f-columns handled per core per expert
FT = FH // P       # 16 f-tiles (partition tiles of h) per phase
DT = D // P        # 8  d-tiles (partition tiles of y)
FBLK = 512         # W1 wave width (f-columns per wave)
FB = FH // FBLK    # 4 waves per phase (8 total)
GO = 8             # fo-tiles per W2 batch (2 batches per phase, 4 total)
WARM_N = 68        # PE warmup matmuls (HAM un-throttle) during startup DMA

# test-harness hooks (left off for grading)
TRACE = False
LAST_RESULTS = None

_compiled = {}


def _split_drain_waits(nc, max_waits=1):
    """This walrus build rejects instructions carrying more than one sync
    wait ("Too many sync wait commands"). Keep one wait on the instruction and
    move the excess onto NoOps inserted right before it on the same engine
    (engines are in-order, so blocking semantics are identical). Updates stay
    on the original instruction — moving them to a trailing NoOp could signal
    before the op's writes land."""
    import concourse.mybir as mybir

    m = nc.m
    new_module = copy.replace(m, functions=[])
    for function in m.functions:
        new_function = copy.replace(function, blocks=[])
        new_function.set_allocations_from_list(function.allocations)
        for block in function.blocks:
            out = []
            for inst in block.instructions:
                si = getattr(inst, "sync_info", None)
                on_wait = list(si.on_wait) if si is not None and si.on_wait else []
                if len(on_wait) > max_waits:
                    engine = getattr(inst, "engine", None)
                    extra, keep = on_wait[max_waits:], on_wait[:max_waits]
                    for j, w in enumerate(extra):
                        out.append(
                            mybir.InstNoOp(
                                name=f"{inst.name}-w{j}",
                                engine=engine,
                                sync_info=mybir.SyncInfo(on_wait=[w], on_update=[]),
                                bass_nofuse=True,
                            )
                        )
                    inst.sync_info = mybir.SyncInfo(
                        on_wait=keep,
                        on_update=list(si.on_update) if si.on_update else [],
                    )
                out.append(inst)
            new_function.blocks.append(copy.replace(block, instructions=out))
        new_module.functions.append(new_function)
    nc.m = new_module
    return nc


def _chunk_list(CA, CB):
    """Global chunk descriptors: (phase, c0_in_phase, cn)."""
    chunks = []
    for ph, C in ((0, CA), (1, CB)):
        c0 = 0
        while c0 < C:
            chunks.append((ph, c0, min(CC, C - c0)))
            c0 += CC
    return chunks


def _build_nc(CA, CB):
    import concourse.bass as bass
    import concourse.mybir as mybir
    from concourse.tile import TileContext

    fp32 = mybir.dt.float32
    bf16 = mybir.dt.bfloat16
    AF = mybir.ActivationFunctionType

    chunks = _chunk_list(CA, CB)
    nch = len(chunks)

    nc = bass.Bass()
    # All DRAM tensors are packed host-side so that one SBUF partition's data
    # is contiguous in DRAM (8-16KB per partition per transfer).
    # Waves 0-3 / W2 batches 0-1 / b-cols 0-15 belong to expert phase A;
    # waves 4-7 / batches 2-3 / b-cols 16-31 to phase B.
    xp = nc.declare_dram_parameter("xp", [nch, P, KO * CC], bf16, isOutput=False)
    w1p = nc.declare_dram_parameter("w1p", [2 * FB, P, KO * FBLK], bf16,
                                    isOutput=False)
    w2p = nc.declare_dram_parameter("w2p", [4, P, GO * D], bf16, isOutput=False)
    b1 = nc.declare_dram_parameter("b1", [P, 2 * FT], fp32, isOutput=False)
    b2 = nc.declare_dram_parameter("b2", [P, 2 * DT], fp32, isOutput=False)
    yp = nc.declare_dram_parameter("yp", [nch, P, DT * CC], bf16, isOutput=True)

    with TileContext(nc) as tc:
        with (
            tc.tile_pool(name="wpool", bufs=1) as wpool,
            tc.tile_pool(name="xpool", bufs=2) as xpool,
            tc.tile_pool(name="hpool", bufs=1) as hpool,
            tc.tile_pool(name="ypool", bufs=2) as ypool,
            tc.tile_pool(name="hpsum", bufs=4, space="PSUM") as hpsum,
            tc.tile_pool(name="ypsum", bufs=4, space="PSUM") as ypsum,
        ):
            # The warm tile is memset on-device so the PE warmup loop can
            # start during the DMA ramp without waiting on any transfer.
            warm_sb = wpool.tile([P, P], bf16, tag="warm")
            nc.vector.memset(warm_sb[:], 0.0)

            # DMA ring order == program order: chunk-0 x and W1-wave-0 arrive
            # in ko-pair quarters so the first mm1 chain can start after only
            # ~0.5MB has landed; then the rest in deadline order.
            x_sb = [None] * nch
            x_sb[0] = xpool.tile([P, KO * CC], bf16, tag="x", name="x0")
            w1_sb = [None] * (2 * FB)
            w1_sb[0] = wpool.tile([P, KO * FBLK], bf16, tag="w1w0", name="w1s0")
            for j in range(4):
                nc.sync.dma_start(
                    x_sb[0][:, 2 * j * CC: 2 * (j + 1) * CC],
                    xp[0, :, 2 * j * CC: 2 * (j + 1) * CC])
                nc.sync.dma_start(
                    w1_sb[0][:, 2 * j * FBLK: 2 * (j + 1) * FBLK],
                    w1p[0, :, 2 * j * FBLK: 2 * (j + 1) * FBLK])
                if j == 0:
                    b1_sb = wpool.tile([P, 2 * FT], fp32, tag="b1")
                    nc.sync.dma_start(b1_sb[:], b1[:])

            b2_sb = wpool.tile([P, 2 * DT], fp32, tag="b2")
            nc.sync.dma_start(b2_sb[:], b2[:])

            # Deadline order: phase-A W1 waves, phase-A W2 batches, then all
            # of phase B (needed only ~235us in).
            w2_sb = [None] * 4
            for fb in list(range(1, FB)) + [None] + list(range(FB, 2 * FB)):
                if fb is None:
                    for g in (0, 1):
                        w2_sb[g] = wpool.tile([P, GO * D], bf16, tag=f"w2g{g}",
                                              name=f"w2s{g}")
                        nc.sync.dma_start(w2_sb[g][:], w2p[g])
                    continue
                w1_sb[fb] = wpool.tile([P, KO * FBLK], bf16, tag=f"w1w{fb}",
                                       name=f"w1s{fb}")
                if fb < FB:
                    # phase-one waves in halves: finer arrival granularity
                    # while the DMA engines are still ramping
                    HW = KO // 2 * FBLK
                    nc.sync.dma_start(w1_sb[fb][:, :HW], w1p[fb, :, :HW])
                    nc.sync.dma_start(w1_sb[fb][:, HW:], w1p[fb, :, HW:])
                else:
                    nc.sync.dma_start(w1_sb[fb][:], w1p[fb])
            for g in (2, 3):
                w2_sb[g] = wpool.tile([P, GO * D], bf16, tag=f"w2g{g}",
                                      name=f"w2s{g}")
                nc.sync.dma_start(w2_sb[g][:], w2p[g])

            # PE warmup: matmuls on the memset tile (no DMA deps) keep the PE
            # busy through the HAM activity window so the real matmuls start
            # at the full 2.4 GHz clock. Count is paced to end roughly when
            # the chunk-0 activations/weights land.
            warm_ps = hpsum.tile([P, CC], fp32, tag="hps")
            for _ in range(WARM_N):
                nc.tensor.matmul(
                    warm_ps[:, 0:64], warm_sb[:, 0:P], warm_sb[:, 0:64],
                    start=True, stop=True,
                )

            for ci, (ph, c0, cn) in enumerate(chunks):
                if ci + 1 < nch:
                    x_sb[ci + 1] = xpool.tile([P, KO * CC], bf16, tag="x",
                                              name=f"x{ci + 1}")
                    nc.sync.dma_start(x_sb[ci + 1][:], xp[ci + 1])

                h_sb = hpool.tile([P, FT * CC], bf16, tag="h")
                for ft in range(FT):
                    fb, fc = divmod(ft * P, FBLK)
                    fb += ph * FB
                    h_ps = hpsum.tile([P, CC], fp32, tag="hps")
                    for ko in range(KO):
                        nc.tensor.matmul(
                            h_ps[:, :cn],
                            w1_sb[fb][:, ko * FBLK + fc: ko * FBLK + fc + P],
                            x_sb[ci][:, ko * CC: ko * CC + cn],
                            start=(ko == 0),
                            stop=(ko == KO - 1),
                        )
                    # gelu(mm + b1) fused on ScalarE, cast to bf16 on write
                    nc.scalar.activation(
                        h_sb[:, ft * CC: ft * CC + cn], h_ps[:, :cn], AF.Gelu,
                        bias=b1_sb[:, ph * FT + ft: ph * FT + ft + 1],
                    )

                y_sb = ypool.tile([P, DT * CC], bf16, tag="y")
                for dt_ in range(DT):
                    y_ps = ypsum.tile([P, CC], fp32, tag="yps")
                    for fo in range(FT):
                        g = 2 * ph + fo // GO
                        gl = fo % GO
                        nc.tensor.matmul(
                            y_ps[:, :cn],
                            w2_sb[g][:, gl * D + dt_ * P: gl * D + dt_ * P + P],
                            h_sb[:, fo * CC: fo * CC + cn],
                            start=(fo == 0),
                            stop=(fo == FT - 1),
                        )
                    nc.vector.tensor_scalar_add(
                        y_sb[:, dt_ * CC: dt_ * CC + cn], y_ps[:, :cn],
                        b2_sb[:, ph * DT + dt_: ph * DT + dt_ + 1],
                    )
                if cn == CC and ci + 1 < nch:
                    nc.sync.dma_start(yp[ci], y_sb[:])
                else:
                    # Partial chunks and the final chunk drain per d-tile so
                    # the output DMA starts as soon as each DVE add lands.
                    for dt_ in range(DT):
                        nc.sync.dma_start(
                            yp[ci, :, dt_ * CC: dt_ * CC + cn],
                            y_sb[:, dt_ * CC: dt_ * CC + cn],
                        )

    return _split_drain_waits(nc)


def _to_bf16(a):
    """Fast float32 -> bfloat16 with round-to-nearest-even via bit ops."""
    a = np.ascontiguousarray(a, dtype=np.float32)
    u = a.view(np.uint32)
    r = ((u + 0x7FFF + ((u >> 16) & 1)) >> 16).astype(np.uint16)
    return r.view(ml_dtypes.bfloat16)


def _pack_x(hs_rows, C, nchp):
    """[cnt, D] tokens -> [nchp, P, KO*CC] zero-padded chunk tiles."""
    xT = np.zeros((D, nchp * CC), dtype=ml_dtypes.bfloat16)
    xT[:, :hs_rows.shape[0]] = _to_bf16(hs_rows).T
    return xT.reshape(KO, P, nchp, CC).transpose(2, 1, 0, 3).reshape(
        nchp, P, KO * CC)


def _pack_w1_half(W1e, h):
    """W1[e] [D, F] F-half h -> [FB, P, KO*FBLK] wave tiles."""
    half = np.ascontiguousarray(W1e[:, h * FH:(h + 1) * FH])
    return np.ascontiguousarray(
        _to_bf16(half).reshape(KO, P, FB, FBLK)
        .transpose(2, 1, 0, 3).reshape(FB, P, KO * FBLK))


def _pack_w2_half(W2e, h):
    """W2[e] [F, D] row-half h -> [2, P, GO*D] batch tiles."""
    half = np.ascontiguousarray(W2e[h * FH:(h + 1) * FH, :])
    return np.ascontiguousarray(
        _to_bf16(half).reshape(2, GO, P, D)
        .transpose(0, 2, 1, 3).reshape(2, P, GO * D))


def kernel(hidden_states, Wg, bg, W1, b1, W2, b2):
    from concourse import bass_utils

    hs = np.ascontiguousarray(hidden_states, dtype=np.float32).reshape(B * S, D)

    # ---- Gate on host (float64): softmax over experts, top-2, renormalize
    logits = hs.astype(np.float64) @ np.asarray(Wg, np.float64).T
    logits += np.asarray(bg, np.float64)
    logits -= logits.max(axis=-1, keepdims=True)
    p = np.exp(logits)
    p /= p.sum(axis=-1, keepdims=True)

    i1 = p.argmax(axis=-1)
    rows = np.arange(B * S)
    p1 = p[rows, i1]
    pm = p.copy()
    pm[rows, i1] = -1.0
    i2 = pm.argmax(axis=-1)
    p2 = p[rows, i2]
    denom = p1 + p2
    g1 = (p1 / denom).astype(np.float32)
    g2 = (p2 / denom).astype(np.float32)

    # ---- Dispatch: token ids + combine weights per expert
    ids, cws = [], []
    for e in range(E):
        m1 = np.nonzero(i1 == e)[0]
        m2 = np.nonzero(i2 == e)[0]
        ids.append(np.concatenate([m1, m2]))
        cws.append(np.concatenate([g1[m1], g2[m2]]))

    # Slot A = the 4 least busy experts, slot B = the 4 busiest; pair A_p
    # with B_p. Phase capacities are the max count within each slot. The
    # busy slot runs second so the program ends on its small partial chunk
    # (minimal final output drain).
    order = sorted(range(E), key=lambda e: -len(ids[e]))
    slotA, slotB = order[4:], order[:4]
    CA = max(P, -(-max(len(ids[e]) for e in slotA) // P) * P)
    CB = max(P, -(-max(len(ids[e]) for e in slotB) // P) * P)
    nchA, nchB = -(-CA // CC), -(-CB // CC)
    nch = nchA + nchB

    if (CA, CB) not in _compiled:
        _compiled[(CA, CB)] = _build_nc(CA, CB)
    nc = _compiled[(CA, CB)]

    in_maps = []
    for pr in range(4):
        ea, eb = slotA[pr], slotB[pr]
        xpk = np.concatenate(
            [_pack_x(hs[ids[ea]], CA, nchA), _pack_x(hs[ids[eb]], CB, nchB)])
        for h in range(2):
            w1pk = np.concatenate(
                [_pack_w1_half(np.asarray(W1[ea]), h),
                 _pack_w1_half(np.asarray(W1[eb]), h)])
            w2pk = np.concatenate(
                [_pack_w2_half(np.asarray(W2[ea]), h),
                 _pack_w2_half(np.asarray(W2[eb]), h)])
            b1pk = np.concatenate(
                [np.asarray(b1[ea], np.float32)[h * FH:(h + 1) * FH]
                 .reshape(FT, P).T,
                 np.asarray(b1[eb], np.float32)[h * FH:(h + 1) * FH]
                 .reshape(FT, P).T], axis=1)
            b2pk = np.concatenate(
                [np.asarray(b2[ea], np.float32).reshape(DT, P).T * 0.5,
                 np.asarray(b2[eb], np.float32).reshape(DT, P).T * 0.5], axis=1)
            in_maps.append({
                "xp": xpk,
                "w1p": w1pk,
                "w2p": w2pk,
                "b1": np.ascontiguousarray(b1pk),
                "b2": np.ascontiguousarray(b2pk),
            })

    kwargs = {}
    if TRACE:
        import os as _os
        kwargs = dict(trace=True, trace_cores=list(range(E)))
        if _os.environ.get("MOE_TRACE_DIR"):
            _os.makedirs(_os.environ["MOE_TRACE_DIR"], exist_ok=True)
            kwargs["tmpdir"] = _os.environ["MOE_TRACE_DIR"]
    res = bass_utils.run_bass_kernel_spmd(nc, in_maps, list(range(E)), **kwargs)
    global LAST_RESULTS
    LAST_RESULTS = res

    def unpack(ypk, lo, nchp):
        # yp[ch, p, dt*CC + c'] = y[dt*P + p, ch*CC + c']
        blk = ypk[lo:lo + nchp]
        return blk.reshape(nchp, P, DT, CC).transpose(2, 1, 0, 3).reshape(
            D, nchp * CC)

    out = np.zeros((B * S, D), dtype=np.float32)
    for pr in range(4):
        y0 = np.asarray(res.results[2 * pr]["yp"], dtype=np.float32)
        y1 = np.asarray(res.results[2 * pr + 1]["yp"], dtype=np.float32)
        for (e, lo, nchp) in ((slotA[pr], 0, nchA), (slotB[pr], nchA, nchB)):
            cnt = len(ids[e])
            yT = unpack(y0, lo, nchp) + unpack(y1, lo, nchp)
            out[ids[e]] += cws[e][:, None] * yT[:, :cnt].T
    return out.reshape(B, S, D)
